# revision 1
# baseline (speedup 1.0000x reference)
"""Trainium2 Bass kernel for nn_Loss_65781719105930 (YOLO-style detection loss).

Strategy (pure data parallelism, 8 cores, 32 images each):
  host:   replicate the reference's target-build scatter (small int64 inputs),
          derive per-occupied-cell aux tables; gather occupied-cell prediction
          columns; shard everything by image.
  device: dense pass over the 5 conf channels (sum of sigmoid^2 — the only
          term every cell contributes to), plus the full IoU / first-argmax /
          best-anchor-select / cross-entropy math on compacted occupied-cell
          tiles.  Per-core partial sums come back; host combines and scales.

The grid offset cancels algebraically in both the IoU and the box loss, so it
never appears on device.
"""
import numpy as np

# ---------------------------------------------------------------- constants
NCLS = 20
H = W = 32
HWC = H * W            # 1024 cells/image
A = 5
M = 50
B = 256
CORES = 8
BC = B // CORES        # 32 images per core
CH = A * (5 + NCLS)    # 125 channels
P = 128
T = 13                 # cell blocks per partition -> 128*13 = 1664 slots/core
SLOTS = P * T
LAM_COORD, LAM_OBJ, LAM_NOOBJ, LAM_CLS = 5.0, 1.0, 0.5, 1.0

_CACHE = {}


# ---------------------------------------------------------------- host prep
def _build_target_np(gt_boxes, gt_classes, num_box):
    """Numpy replication of reference.build_target (last object wins, first-max
    class argmax). Returns per-cell [B, HWC] arrays."""
    Bn = gt_boxes.shape[0]
    valid = np.arange(M)[None, :] < num_box[:, None]
    x = gt_boxes[..., 0].astype(np.float32) * H
    y = gt_boxes[..., 1].astype(np.float32) * H
    gx = np.floor(x).astype(np.int64)
    gy = np.floor(y).astype(np.int64)
    flat = np.where(valid, gy * W + gx, HWC)
    bi = np.broadcast_to(np.arange(Bn)[:, None], (Bn, M))

    vals = np.stack([np.ones_like(x), x - gx, y - gy,
                     gt_boxes[..., 2].astype(np.float32) * H,
                     gt_boxes[..., 3].astype(np.float32) * H], axis=-1)
    tgt_box = np.zeros((Bn, HWC + 1, 5), dtype=np.float32)
    tgt_box[bi, flat] = vals
    tgt_cls = np.zeros((Bn, HWC + 1, NCLS), dtype=np.float32)
    tgt_cls[bi, flat, gt_classes.astype(np.int64)] = 1.0

    tgt_box = tgt_box[:, :HWC]
    obj = tgt_box[..., 0]
    cls_t = np.argmax(tgt_cls[:, :HWC], axis=-1).astype(np.int32)
    return obj, tgt_box[..., 1], tgt_box[..., 2], tgt_box[..., 3], tgt_box[..., 4], cls_t


def _split_multi_waits(nc):
    """This container's walrus accepts only ONE sem-wait per instruction; hoist
    extra waits onto standalone NoOps."""
    import concourse.mybir as mybir
    import bass_rust
    n = 0
    for fn in nc.m.functions:
        for blk in fn.blocks:
            new = []
            for ins in blk.instructions:
                si = ins.sync_info
                waits = list(si.on_wait) if si is not None else []
                if len(waits) > 1:
                    for w in waits[:-1]:
                        nop = mybir.InstNoOp(name=f"{ins.name}-w{n}")
                        nop.engine = ins.engine
                        nop.sync_info = bass_rust.SyncInfo(on_wait=[w], on_update=[])
                        new.append(nop)
                        n += 1
                    si.on_wait = [waits[-1]]
                    ins.sync_info = si
                new.append(ins)
            blk.instructions = new
    return n


# ---------------------------------------------------------------- bass build
def _build_nc(split=True):
    import concourse.bass as bass
    import concourse.mybir as mybir
    import concourse.tile as tile

    f32 = mybir.dt.float32
    AF = mybir.ActivationFunctionType
    OP = mybir.AluOpType
    AX = mybir.AxisListType

    def _v(ap, off, dims):
        """Sub-view of a tile AP: keep its partition dim, replace free dims."""
        return bass.AP(tensor=ap.tensor, offset=ap.offset + off,
                       ap=[list(ap.ap[0])] + dims)

    nc = bass.Bass("TRN2")
    xout = nc.declare_dram_parameter("xout", [BC * CH, HWC], f32, isOutput=False)
    cols = nc.declare_dram_parameter("cols", [P, T * CH], f32, isOutput=False)
    aux13 = nc.declare_dram_parameter("aux13", [P, 5 * T], f32, isOutput=False)
    aux65 = nc.declare_dram_parameter("aux65", [P, 6 * 65], f32, isOutput=False)
    ahalf = nc.declare_dram_parameter("ahalf", [P, 130], f32, isOutput=False)
    onehot = nc.declare_dram_parameter("onehot", [P, T * NCLS], f32, isOutput=False)
    partials_d = nc.declare_dram_parameter("partials", [P, 8], f32, isOutput=True)

    with tile.TileContext(nc) as tc:
        with tc.tile_pool(name="sb", bufs=1) as pool:
            # ---------------- dense conf pass: sum over all cells of sigmoid^2
            xa = xout[:]
            conf_src1 = bass.AP(tensor=xa.tensor, offset=20 * HWC,
                                ap=[[CH * HWC, 25], [25 * HWC, A], [1, HWC]])
            conf_src2 = bass.AP(tensor=xa.tensor, offset=25 * CH * HWC + 20 * HWC,
                                ap=[[CH * HWC, BC - 25], [25 * HWC, A], [1, HWC]])
            tc1 = pool.tile([125, HWC], f32, name="tc1")
            tc2 = pool.tile([(BC - 25) * A, HWC], f32, name="tc2")
            nc.sync.dma_start(out=tc1[:], in_=conf_src1)
            nc.sync.dma_start(out=tc2[:], in_=conf_src2)

            partials = pool.tile([P, 8], f32, name="partials")
            nc.vector.memset(partials[:], 0.0)

            sg1 = pool.tile([125, HWC], f32, name="sg1")
            sg2 = pool.tile([(BC - 25) * A, HWC], f32, name="sg2")
            nc.scalar.activation(sg1[:], tc1[:], AF.Sigmoid)
            nc.scalar.activation(sg2[:], tc2[:], AF.Sigmoid)
            sq1 = pool.tile([125, HWC], f32, name="sq1")
            sq2 = pool.tile([(BC - 25) * A, HWC], f32, name="sq2")
            acc1 = bass.AP(tensor=partials[:].tensor, offset=partials[:].offset + 4,
                           ap=[[8, 125], [1, 1]])
            acc2 = bass.AP(tensor=partials[:].tensor, offset=partials[:].offset + 5,
                           ap=[[8, (BC - 25) * A], [1, 1]])
            nc.scalar.activation(sq1[:], sg1[:], AF.Square, accum_out=acc1)
            nc.scalar.activation(sq2[:], sg2[:], AF.Square, accum_out=acc2)

            # ---------------- sparse inputs
            raw = pool.tile([P, T * CH], f32, name="raw")
            nc.sync.dma_start(out=raw[:], in_=cols[:])
            a13 = pool.tile([P, 5 * T], f32, name="a13")
            nc.sync.dma_start(out=a13[:], in_=aux13[:])
            a65 = pool.tile([P, 6 * 65], f32, name="a65")
            nc.sync.dma_start(out=a65[:], in_=aux65[:])
            ah = pool.tile([P, 130], f32, name="ah")
            nc.sync.dma_start(out=ah[:], in_=ahalf[:])
            oh = pool.tile([P, T * NCLS], f32, name="oh")
            nc.sync.dma_start(out=oh[:], in_=onehot[:])

            r = raw[:]
            OBJ = _v(a13[:], 0 * T, [[1, T]])
            XO = _v(a13[:], 1 * T, [[1, T]])
            YO = _v(a13[:], 2 * T, [[1, T]])
            SQTW = _v(a13[:], 3 * T, [[1, T]])
            SQTH = _v(a13[:], 4 * T, [[1, T]])
            # (t, a)-flat planes, used against dense-65 operands
            BX1 = _v(a65[:], 0 * 65, [[1, 65]])
            BX2 = _v(a65[:], 1 * 65, [[1, 65]])
            BY1 = _v(a65[:], 2 * 65, [[1, 65]])
            BY2 = _v(a65[:], 3 * 65, [[1, 65]])
            TAREA = _v(a65[:], 4 * 65, [[1, 65]])
            WCONST = _v(a65[:], 5 * 65, [[1, 65]])

            tcnt = [0]

            def t65():
                tcnt[0] += 1
                return pool.tile([P, 65], f32, name=f"t65_{tcnt[0]}")

            def t13():
                tcnt[0] += 1
                return pool.tile([P, T], f32, name=f"t13_{tcnt[0]}")

            def TA(tile_):
                """(t, a)-structured view of a dense [P, 65] tile."""
                return _v(tile_[:], 0, [[A, T], [1, A]])

            # sigmoid(xy), exp(wh)*anchor/2, sigmoid(conf)
            sigxy = pool.tile([P, 130], f32, name="sigxy")
            nc.scalar.activation(_v(sigxy[:], 0, [[10, T], [2, A], [1, 2]]),
                                 _v(r, 21, [[CH, T], [25, A], [1, 2]]), AF.Sigmoid)
            expwh = pool.tile([P, 130], f32, name="expwh")
            nc.scalar.activation(_v(expwh[:], 0, [[10, T], [2, A], [1, 2]]),
                                 _v(r, 23, [[CH, T], [25, A], [1, 2]]), AF.Exp)
            whalf = pool.tile([P, 130], f32, name="whalf")
            nc.vector.tensor_mul(whalf[:], expwh[:], ah[:])
            sigc = pool.tile([P, 65], f32, name="sigc")
            nc.scalar.activation(TA(sigc), _v(r, 20, [[CH, T], [25, A]]), AF.Sigmoid)

            Xv = _v(sigxy[:], 0, [[10, T], [2, A]])
            Yv = _v(sigxy[:], 1, [[10, T], [2, A]])
            WXv = _v(whalf[:], 0, [[10, T], [2, A]])
            WYv = _v(whalf[:], 1, [[10, T], [2, A]])

            # IoU  (all [P, 65] tiles in (t, a)-flat layout)
            ax1 = t65(); nc.vector.tensor_sub(TA(ax1), Xv, WXv)
            ax2 = t65(); nc.vector.tensor_add(TA(ax2), Xv, WXv)
            ay1 = t65(); nc.vector.tensor_sub(TA(ay1), Yv, WYv)
            ay2 = t65(); nc.vector.tensor_add(TA(ay2), Yv, WYv)
            t1 = t65(); nc.vector.tensor_tensor(out=t1[:], in0=ax2[:], in1=BX2, op=OP.min)
            t2 = t65(); nc.vector.tensor_tensor(out=t2[:], in0=ax1[:], in1=BX1, op=OP.max)
            t3 = t65(); nc.vector.tensor_sub(t3[:], t1[:], t2[:])
            iw = t65(); nc.vector.tensor_scalar_max(iw[:], t3[:], 0.0)
            t4 = t65(); nc.vector.tensor_tensor(out=t4[:], in0=ay2[:], in1=BY2, op=OP.min)
            t5 = t65(); nc.vector.tensor_tensor(out=t5[:], in0=ay1[:], in1=BY1, op=OP.max)
            t6 = t65(); nc.vector.tensor_sub(t6[:], t4[:], t5[:])
            ih = t65(); nc.vector.tensor_scalar_max(ih[:], t6[:], 0.0)
            inter = t65(); nc.vector.tensor_mul(inter[:], iw[:], ih[:])
            u1 = t65(); nc.vector.tensor_mul(TA(u1), WXv, WYv)
            u2 = t65(); nc.vector.tensor_scalar_mul(u2[:], u1[:], 4.0)
            u3 = t65(); nc.vector.tensor_add(u3[:], u2[:], TAREA)
            u4 = t65(); nc.vector.tensor_sub(u4[:], u3[:], inter[:])
            u5 = t65(); nc.vector.tensor_scalar_max(u5[:], u4[:], 1e-10)
            rcp = t65(); nc.vector.reciprocal(rcp[:], u5[:])
            iou = t65(); nc.vector.tensor_mul(iou[:], inter[:], rcp[:])

            # first-argmax -> fmask (exact float equality on identical values)
            rmax = t13()
            nc.vector.tensor_reduce(out=rmax[:], in_=TA(iou), axis=AX.X, op=OP.max)
            rmax5 = t65()
            for a in range(A):
                nc.vector.tensor_copy(out=_v(rmax5[:], a, [[A, T]]), in_=rmax[:])
            eq = t65(); nc.vector.tensor_tensor(out=eq[:], in0=iou[:], in1=rmax5[:], op=OP.is_equal)
            fval = t65(); nc.vector.tensor_tensor(out=fval[:], in0=eq[:], in1=WCONST, op=OP.mult)
            m2 = t13()
            nc.vector.tensor_reduce(out=m2[:], in_=TA(fval), axis=AX.X, op=OP.max)
            m25 = t65()
            for a in range(A):
                nc.vector.tensor_copy(out=_v(m25[:], a, [[A, T]]), in_=m2[:])
            fmask = t65()
            nc.vector.tensor_tensor(out=fmask[:], in0=fval[:], in1=m25[:], op=OP.is_equal)

            # best-anchor selections (sum over a of fmask * value)
            def sel(src_view, structured):
                tmp = t65()
                if structured:
                    nc.vector.tensor_tensor(out=TA(tmp), in0=TA(fmask), in1=src_view, op=OP.mult)
                else:
                    nc.vector.tensor_tensor(out=tmp[:], in0=fmask[:], in1=src_view, op=OP.mult)
                out = t13()
                nc.vector.tensor_reduce(out=out[:], in_=TA(tmp), axis=AX.X, op=OP.add)
                return out

            px = sel(Xv, True)
            py = sel(Yv, True)
            pwh = sel(WXv, True)   # half-width; Sqrt uses scale=2
            phh = sel(WYv, True)
            cb = sel(sigc[:], False)

            # box loss
            dx = t13(); nc.vector.tensor_sub(dx[:], px[:], XO)
            dx2 = t13(); nc.vector.tensor_mul(dx2[:], dx[:], dx[:])
            dy = t13(); nc.vector.tensor_sub(dy[:], py[:], YO)
            dy2 = t13(); nc.vector.tensor_mul(dy2[:], dy[:], dy[:])
            sqw = t13(); nc.scalar.activation(sqw[:], pwh[:], AF.Sqrt, scale=2.0)
            dw = t13(); nc.vector.tensor_sub(dw[:], sqw[:], SQTW)
            dw2 = t13(); nc.vector.tensor_mul(dw2[:], dw[:], dw[:])
            sqh = t13(); nc.scalar.activation(sqh[:], phh[:], AF.Sqrt, scale=2.0)
            dh = t13(); nc.vector.tensor_sub(dh[:], sqh[:], SQTH)
            dh2 = t13(); nc.vector.tensor_mul(dh2[:], dh[:], dh[:])
            s1 = t13(); nc.vector.tensor_add(s1[:], dx2[:], dy2[:])
            s2 = t13(); nc.vector.tensor_add(s2[:], dw2[:], dh2[:])
            s3 = t13(); nc.vector.tensor_add(s3[:], s1[:], s2[:])
            boxc = t13(); nc.vector.tensor_tensor(out=boxc[:], in0=s3[:], in1=OBJ, op=OP.mult)
            nc.vector.tensor_reduce(out=_v(partials[:], 0, [[1, 1]]), in_=boxc[:],
                                    axis=AX.X, op=OP.add)

            # conf loss + noobj correction
            cbm = t13(); nc.vector.tensor_scalar_add(cbm[:], cb[:], -1.0)
            cbm2 = t13(); nc.vector.tensor_mul(cbm2[:], cbm[:], cbm[:])
            confc = t13(); nc.vector.tensor_tensor(out=confc[:], in0=cbm2[:], in1=OBJ, op=OP.mult)
            nc.vector.tensor_reduce(out=_v(partials[:], 1, [[1, 1]]), in_=confc[:],
                                    axis=AX.X, op=OP.add)
            cb2 = t13(); nc.vector.tensor_mul(cb2[:], cb[:], cb[:])
            nobc = t13(); nc.vector.tensor_tensor(out=nobc[:], in0=cb2[:], in1=OBJ, op=OP.mult)
            nc.vector.tensor_reduce(out=_v(partials[:], 2, [[1, 1]]), in_=nobc[:],
                                    axis=AX.X, op=OP.add)

            # cls loss: logsumexp (logits ~ N(0,1), no max-sub needed) - picked logit
            e = pool.tile([P, A * T * NCLS], f32, name="e")   # (a, t, j)
            for a in range(A):
                nc.scalar.activation(_v(e[:], a * T * NCLS, [[NCLS, T], [1, NCLS]]),
                                     _v(r, a * 25, [[CH, T], [1, NCLS]]), AF.Exp)
            se = t65()   # (a, t)-flat
            nc.vector.tensor_reduce(out=se[:],
                                    in_=_v(e[:], 0, [[T * NCLS, A], [NCLS, T], [1, NCLS]]),
                                    axis=AX.X, op=OP.add)
            lg = t65(); nc.scalar.activation(lg[:], se[:], AF.Ln)    # (a, t)
            s = t65()    # (a, t)
            for a in range(A):
                stmp = pool.tile([P, T * NCLS], f32, name=f"stmp{a}")
                nc.vector.tensor_tensor(out=stmp[:], in0=_v(r, a * 25, [[CH, T], [1, NCLS]]),
                                        in1=_v(oh[:], 0, [[NCLS, T], [1, NCLS]]), op=OP.mult)
                nc.vector.tensor_reduce(out=_v(s[:], a * T, [[1, T]]),
                                        in_=_v(stmp[:], 0, [[NCLS, T], [1, NCLS]]),
                                        axis=AX.X, op=OP.add)
            ce = t65(); nc.vector.tensor_sub(ce[:], lg[:], s[:])     # (a, t)-flat
            mce = t65()  # (t, a)-flat
            nc.vector.tensor_tensor(out=TA(mce),
                                    in0=_v(ce[:], 0, [[1, T], [T, A]]),
                                    in1=TA(fmask), op=OP.mult)
            cls13 = t13()
            nc.vector.tensor_reduce(out=cls13[:], in_=TA(mce), axis=AX.X, op=OP.add)
            clsc = t13(); nc.vector.tensor_tensor(out=clsc[:], in0=cls13[:], in1=OBJ, op=OP.mult)
            nc.vector.tensor_reduce(out=_v(partials[:], 3, [[1, 1]]), in_=clsc[:],
                                    axis=AX.X, op=OP.add)

            nc.sync.dma_start(out=partials_d[:], in_=partials[:])

    if split:
        _split_multi_waits(nc)
    return nc


# -------------------------------------------------------------- shard builder
def _make_in_maps(out, gt_boxes, anchor_np, gt_classes_np, num_box_np):
    obj, xo, yo, tw, th, cls_t = _build_target_np(gt_boxes, gt_classes_np, num_box_np)
    out_r = out.reshape(B, CH, HWC)

    in_maps = []
    for c in range(CORES):
        sl = slice(c * BC, (c + 1) * BC)
        ob = obj[sl]                       # [BC, HWC]
        bloc, hwloc = np.nonzero(ob > 0)
        K = len(bloc)
        assert K <= SLOTS

        def place(vals):
            buf = np.zeros(SLOTS, dtype=np.float32)
            buf[:K] = vals
            return buf.reshape(P, T)

        objv = place(np.ones(K, dtype=np.float32))
        xov = place(xo[sl][bloc, hwloc])
        yov = place(yo[sl][bloc, hwloc])
        twv = place(tw[sl][bloc, hwloc])
        thv = place(th[sl][bloc, hwloc])
        clsv = place(cls_t[sl][bloc, hwloc]).astype(np.int32)

        aux13 = np.concatenate(
            [objv, xov, yov, np.sqrt(twv), np.sqrt(thv)], axis=1)      # [P, 5T]

        bx1 = xov - twv * 0.5; bx2 = xov + twv * 0.5
        by1 = yov - thv * 0.5; by2 = yov + thv * 0.5
        tarea = twv * thv
        wconst = np.broadcast_to((A - np.arange(A, dtype=np.float32)), (P, T, A))

        def rep(x):                        # [P, T] -> [P, T*A] in (t, a) layout
            return np.repeat(x[:, :, None], A, axis=2).reshape(P, T * A)

        aux65 = np.concatenate(
            [rep(bx1), rep(bx2), rep(by1), rep(by2), rep(tarea),
             np.ascontiguousarray(wconst).reshape(P, T * A)], axis=1)  # [P, 390]

        ahalf = np.ascontiguousarray(
            np.broadcast_to(anchor_np[None, None] * 0.5,
                            (P, T, A, 2))).reshape(P, T * A * 2)

        onehot = np.zeros((P, T, NCLS), dtype=np.float32)
        pp, tt = np.unravel_index(np.arange(SLOTS), (P, T))
        onehot[pp, tt, clsv[pp, tt]] = 1.0
        onehot = onehot.reshape(P, T * NCLS)

        # host gather of occupied-cell prediction columns [K, CH]
        colsb = np.zeros((SLOTS, CH), dtype=np.float32)
        if K:
            colsb[:K] = out_r[sl][bloc, :, hwloc]
        colsb = colsb.reshape(P, T * CH)

        in_maps.append({
            "xout": np.ascontiguousarray(out_r[sl].reshape(BC * CH, HWC)),
            "cols": np.ascontiguousarray(colsb),
            "aux13": np.ascontiguousarray(aux13),
            "aux65": np.ascontiguousarray(aux65),
            "ahalf": np.ascontiguousarray(ahalf),
            "onehot": np.ascontiguousarray(onehot),
        })
    return in_maps


def _combine(results):
    box_s = conf_s = nob_c = cls_s = dense = 0.0
    for c in range(CORES):
        pr = results[c]["partials"].astype(np.float64)
        box_s += pr[:, 0].sum()
        conf_s += pr[:, 1].sum()
        nob_c += pr[:, 2].sum()
        cls_s += pr[:, 3].sum()
        dense += pr[:125, 4].sum() + pr[:(BC - 25) * A, 5].sum()
    box_loss = np.float32(LAM_COORD / B * box_s)
    conf_loss = np.float32(LAM_OBJ / B * conf_s)
    noobj_loss = np.float32(LAM_NOOBJ / B * (dense - nob_c))
    cls_loss = np.float32(LAM_CLS / B * cls_s)
    return (box_loss, conf_loss, noobj_loss, cls_loss)


# ---------------------------------------------------------------- entry point
def kernel(out, gt_boxes, anchor, gt_classes, num_box):
    from concourse.bass_utils import run_bass_kernel_spmd

    out = np.ascontiguousarray(np.asarray(out, dtype=np.float32))
    gt_boxes = np.asarray(gt_boxes, dtype=np.float32)
    anchor_np = np.asarray(anchor, dtype=np.float32)
    in_maps = _make_in_maps(out, gt_boxes, anchor_np,
                            np.asarray(gt_classes), np.asarray(num_box))

    import os
    if "nc" not in _CACHE:
        _CACHE["nc"] = _build_nc()
    trace = os.environ.get("KERNEL_TRACE", "0") == "1"
    res = run_bass_kernel_spmd(_CACHE["nc"], in_maps, core_ids=list(range(CORES)),
                               trace=trace)
    if trace:
        print(f"HW exec time: {res.exec_time_ns} ns  (mean {res.mean_exec_time_ns})")
    return _combine(res.results)



# revision 14
# speedup vs baseline: 1.8213x; 1.8213x over previous
"""Trainium2 Bass kernel for nn_Loss_65781719105930 (YOLO-style detection loss).

Strategy (pure data parallelism, 8 cores, 32 images each):
  host:   replicate the reference's target-build scatter (small int64 inputs),
          compact occupied cells, pre-pack aux tables + prediction columns into
          three contiguous DMA payloads; gather the target-class logit per
          (cell, anchor) host-side.
  device: dense pass over the 5 conf channels (sum of sigmoid^2), plus IoU /
          first-argmax / best-anchor-select / cross-entropy on compacted tiles.

Numeric tricks that keep the scalar engine on ONE activation-table set
(exp_and_others = {tanh, exp, square}):
  sigmoid(x)   = (1 + tanh(x/2)) / 2      -> work in xi = 2x-1 coords, the
                                             0.5 factors fold into host consts
  sqrt(exp(x)*anchor) = exp(x/2)*sqrt(anchor)
  ln(x)        ~ bitcast_i32(x) * ln2/2^23 - 126.94269504*ln2   (abs err ~2e-2
                 worst, mean-centered; loss tolerance is 2e-2 relative)

The grid offset cancels algebraically in both the IoU and the box loss, so it
never appears on device.
"""
import numpy as np

# ---------------------------------------------------------------- constants
NCLS = 20
H = W = 32
HWC = H * W            # 1024 cells/image
A = 5
M = 50
B = 256
CORES = 8
BC = B // CORES        # 32 images per core
CH = A * (5 + NCLS)    # 125 channels
P = 128
LAM_COORD, LAM_OBJ, LAM_NOOBJ, LAM_CLS = 5.0, 1.0, 0.5, 1.0

LN2 = float(np.log(2.0))
LOG_BIAS = 126.94269504   # mean-centering constant for the log2 bit trick

_CACHE = {}


def _bf16(x):
    """float32 ndarray -> ml_dtypes.bfloat16 (RNE)."""
    import ml_dtypes
    return np.asarray(x, dtype=np.float32).astype(ml_dtypes.bfloat16)


# ---------------------------------------------------------------- host prep
def _build_target_np(gt_boxes, gt_classes, num_box):
    """Numpy replication of reference.build_target (last object wins, first-max
    class argmax). Returns per-cell [B, HWC] arrays."""
    Bn = gt_boxes.shape[0]
    valid = np.arange(M)[None, :] < num_box[:, None]
    x = gt_boxes[..., 0].astype(np.float32) * H
    y = gt_boxes[..., 1].astype(np.float32) * H
    gx = np.floor(x).astype(np.int64)
    gy = np.floor(y).astype(np.int64)
    flat = np.where(valid, gy * W + gx, HWC)
    bi = np.broadcast_to(np.arange(Bn)[:, None], (Bn, M))

    vals = np.stack([np.ones_like(x), x - gx, y - gy,
                     gt_boxes[..., 2].astype(np.float32) * H,
                     gt_boxes[..., 3].astype(np.float32) * H], axis=-1)
    tgt_box = np.zeros((Bn, HWC + 1, 5), dtype=np.float32)
    tgt_box[bi, flat] = vals
    tgt_cls = np.zeros((Bn, HWC + 1, NCLS), dtype=np.float32)
    tgt_cls[bi, flat, gt_classes.astype(np.int64)] = 1.0

    tgt_box = tgt_box[:, :HWC]
    obj = tgt_box[..., 0]
    cls_t = np.argmax(tgt_cls[:, :HWC], axis=-1).astype(np.int64)
    return obj, tgt_box[..., 1], tgt_box[..., 2], tgt_box[..., 3], tgt_box[..., 4], cls_t


def _split_multi_waits(nc):
    """This container's walrus accepts only ONE sem-wait per instruction; hoist
    extra waits onto standalone NoOps."""
    import concourse.mybir as mybir
    import bass_rust
    n = 0
    for fn in nc.m.functions:
        for blk in fn.blocks:
            new = []
            for ins in blk.instructions:
                si = ins.sync_info
                waits = list(si.on_wait) if si is not None else []
                if len(waits) > 1:
                    for w in waits[:-1]:
                        nop = mybir.InstNoOp(name=f"{ins.name}-w{n}")
                        nop.engine = ins.engine
                        nop.sync_info = bass_rust.SyncInfo(on_wait=[w], on_update=[])
                        new.append(nop)
                        n += 1
                    si.on_wait = [waits[-1]]
                    ins.sync_info = si
                new.append(ins)
            blk.instructions = new
    return n


# ---------------------------------------------------------------- bass build
def _build_nc(T, split=True):
    """Build the per-core kernel for T cell-blocks per partition (P*T slots).

    SBUF layouts (all [128, n], f32 unless noted):
      fpack [P, 25T + 18+18+T+5+10 + 5T + 4T+4T+T]:
        cols_xw   (t,a,{conf,x,y,w,h})      25T   occupied-cell chans 20..24
        B1        (t,{x,y})                 2T    xi-space target box lo edges
        B2        (t,{x,y})                 2T    xi-space target box hi edges
        TAREA     (t)                       T     tw*th (physical, cell units)
        WCONST    (a)                       5     A - a   (first-argmax tiebreak)
        SQA       (a,{w,h})                 10    sqrt(anchor)
        S_AUX     (t,a)                     5T    target-class logit
        AUX4      (q,t) q in {x,y,w,h}      4T    (2xo-1, 2yo-1, sqrt tw, sqrt th)
        OSCL4     (q,t)                     4T    obj * {.25,.25,1,1}
        OBJ       (t)                       T     obj mask
      lgpack bf16 [P, 100T]: logits (t,a,j)
      confd  bf16 [P, 1280]: all conf channels of all cells (dense noobj pass)
      partials out [P, 8]: 0 box, 1 sum obj*u^2, 2 sum obj*u (u = tanh(conf/2)),
        3 sum obj*ce, 4 dense sum sigmoid(conf)^2
    """
    import concourse.bass as bass
    import concourse.mybir as mybir
    import concourse.tile as tile

    f32 = mybir.dt.float32
    bf16 = mybir.dt.bfloat16
    i32 = mybir.dt.int32
    AF = mybir.ActivationFunctionType
    OP = mybir.AluOpType
    AX = mybir.AxisListType

    TA = T * A           # (t, a) flat size
    TA2 = TA * 2
    NF = 25 * T + (2 * T + 2 * T + T + 5 + 10) + 5 * T + 4 * T + 4 * T + T

    # fpack free-dim offsets
    O_XW = 0
    O_B1 = 25 * T
    O_B2 = O_B1 + 2 * T
    O_TAREA = O_B2 + 2 * T
    O_WCONST = O_TAREA + T
    O_SQA = O_WCONST + 5
    O_SAUX = O_SQA + 10
    O_AUX4 = O_SAUX + 5 * T
    O_OSCL4 = O_AUX4 + 4 * T
    O_OBJ = O_OSCL4 + 4 * T
    assert O_OBJ + T == NF

    def _v(ap, off, dims):
        """Sub-view of a tile AP: keep its partition dim, replace free dims."""
        return bass.AP(tensor=ap.tensor, offset=ap.offset + off,
                       ap=[list(ap.ap[0])] + dims)

    nc = bass.Bass("TRN2")
    fpack_d = nc.declare_dram_parameter("fpack", [P, NF], f32, isOutput=False)
    lgpack_d = nc.declare_dram_parameter("lgpack", [P, 100 * T], bf16, isOutput=False)
    confd_d = nc.declare_dram_parameter("confd", [P, BC * A * HWC // P], bf16,
                                        isOutput=False)
    partials_d = nc.declare_dram_parameter("partials", [P, 8], f32, isOutput=True)

    DF = BC * A * HWC // P   # 1280 dense conf elements per partition

    with tile.TileContext(nc) as tc:
        with tc.tile_pool(name="sb", bufs=1) as pool:
            # ---------------- input DMAs, priority order, all on sync HWDGE
            fp = pool.tile([P, NF], f32, name="fp")
            nc.sync.dma_start(out=fp[:], in_=fpack_d[:])
            lg_in = pool.tile([P, 100 * T], bf16, name="lg_in")
            nc.sync.dma_start(out=lg_in[:], in_=lgpack_d[:])
            confd = pool.tile([P, DF], bf16, name="confd")
            nc.sync.dma_start(out=confd[:], in_=confd_d[:])

            partials = pool.tile([P, 8], f32, name="partials")

            # ---------------- scalar engine program (one act-table set)
            # SRC: (q, t, a) with q in {u, x, y, w, h, ce}; best-anchor
            # selection later works on all six quantities in one mul+reduce.
            SRC = pool.tile([P, 6 * TA], f32, name="SRC")

            # u, xi_x, xi_y = tanh(chan{conf,x,y}/2); chans 0..2 of cols_xw
            nc.scalar.activation(
                _v(SRC[:], 0, [[TA, 3], [A, T], [1, A]]),
                _v(fp[:], O_XW, [[1, 3], [25, T], [5, A]]),
                AF.Tanh, scale=0.5)
            # EW = exp(chan{w,h}/2)
            EW = pool.tile([P, TA2], f32, name="EW")
            nc.scalar.activation(
                _v(EW[:], 0, [[1, 2], [2 * A, T], [2, A]]),
                _v(fp[:], O_XW + 3, [[1, 2], [25, T], [5, A]]),
                AF.Exp, scale=0.5)
            # e = exp(logits), bf16, (t, a, j)
            e = pool.tile([P, 100 * T], bf16, name="e")
            nc.scalar.activation(
                _v(e[:], 0, [[100, T], [NCLS, A], [1, NCLS]]),
                _v(lg_in[:], 0, [[100, T], [NCLS, A], [1, NCLS]]),
                AF.Exp)
            # dense noobj pass: ud = tanh(c/2); sum sigma^2 = 0.25*(N + 2*sum u
            # + sum u^2) -- avoids a bias const AP on the Square activation.
            UD = pool.tile([P, DF], f32, name="UD")
            nc.scalar.activation(UD[:], confd[:], AF.Tanh, scale=0.5,
                                 accum_out=_v(partials[:], 5, [[1, 1]]))
            SQD = pool.tile([P, DF], f32, name="SQD")
            nc.scalar.activation(SQD[:], UD[:], AF.Square,
                                 accum_out=_v(partials[:], 4, [[1, 1]]))

            # ---------------- vector engine program
            tcnt = [0]

            def tmp(n):
                tcnt[0] += 1
                return pool.tile([P, n], f32, name=f"t{tcnt[0]}")

            # sh = EW * sqrt(anchor)  (= sqrt(pred_wh)); into SRC q3,q4
            SH = _v(SRC[:], 3 * TA, [[TA, 2], [A, T], [1, A]])
            nc.vector.tensor_tensor(
                out=SH,
                in0=_v(EW[:], 0, [[1, 2], [2 * A, T], [2, A]]),
                in1=_v(fp[:], O_SQA, [[1, 2], [0, T], [2, A]]),
                op=OP.mult)
            # wfull = sh*sh = pred_wh (xi-space half-width), (d,t,a) [P, 2TA]
            wf = tmp(TA2)
            SH2 = _v(SRC[:], 3 * TA, [[TA, 2], [1, TA]])
            WF = _v(wf[:], 0, [[TA, 2], [1, TA]])
            nc.vector.tensor_tensor(out=WF, in0=SH2, in1=SH2, op=OP.mult)

            # IoU in xi coords. XY = SRC q1,q2 as (d,t,a)
            XY = _v(SRC[:], TA, [[TA, 2], [1, TA]])
            lo = tmp(TA2)
            nc.vector.tensor_tensor(out=lo[:], in0=XY, in1=WF, op=OP.subtract)
            hi = tmp(TA2)
            nc.vector.tensor_tensor(out=hi[:], in0=XY, in1=WF, op=OP.add)
            # target edges, (d,t) broadcast over a -> (d,t,a)
            B1v = _v(fp[:], O_B1, [[T, 2], [1, T], [0, A]])
            B2v = _v(fp[:], O_B2, [[T, 2], [1, T], [0, A]])
            LOv = _v(lo[:], 0, [[TA, 2], [A, T], [1, A]])
            HIv = _v(hi[:], 0, [[TA, 2], [A, T], [1, A]])
            t1 = tmp(TA2)
            nc.vector.tensor_tensor(out=_v(t1[:], 0, [[TA, 2], [A, T], [1, A]]),
                                    in0=HIv, in1=B2v, op=OP.min)
            t2 = tmp(TA2)
            nc.vector.tensor_tensor(out=_v(t2[:], 0, [[TA, 2], [A, T], [1, A]]),
                                    in0=LOv, in1=B1v, op=OP.max)
            t3 = tmp(TA2)
            nc.vector.tensor_tensor(out=t3[:], in0=t1[:], in1=t2[:], op=OP.subtract)
            # iw = max(t3, 0) * 0.5 -> physical overlap widths (d,t,a)
            iwih = tmp(TA2)
            nc.vector.tensor_scalar(out=iwih[:], in0=t3[:], scalar1=0.0,
                                    scalar2=0.5, op0=OP.max, op1=OP.mult)
            inter = tmp(TA)
            nc.vector.tensor_tensor(out=inter[:], in0=_v(iwih[:], 0, [[1, TA]]),
                                    in1=_v(iwih[:], TA, [[1, TA]]), op=OP.mult)
            areaA = tmp(TA)
            nc.vector.tensor_tensor(out=areaA[:], in0=_v(wf[:], 0, [[1, TA]]),
                                    in1=_v(wf[:], TA, [[1, TA]]), op=OP.mult)
            u1 = tmp(TA)
            nc.vector.tensor_tensor(out=_v(u1[:], 0, [[A, T], [1, A]]),
                                    in0=_v(areaA[:], 0, [[A, T], [1, A]]),
                                    in1=_v(fp[:], O_TAREA, [[1, T], [0, A]]),
                                    op=OP.add)
            u2 = tmp(TA)
            nc.vector.tensor_tensor(out=u2[:], in0=u1[:], in1=inter[:],
                                    op=OP.subtract)
            # this container's walrus rejects ISA-level DVE ops (custom
            # reciprocal_approx_fast / tensor_tensor_reduce): "ISA wrong length"
            rcp = tmp(TA)
            nc.vector.reciprocal(out=rcp[:], in_=u2[:])
            iou = tmp(TA)
            nc.vector.tensor_tensor(out=iou[:], in0=inter[:], in1=rcp[:],
                                    op=OP.mult)

            # first-argmax over a -> fmask (exact float equality + tiebreak)
            rmax = tmp(T)
            nc.vector.tensor_reduce(out=rmax[:],
                                    in_=_v(iou[:], 0, [[A, T], [1, A]]),
                                    axis=AX.X, op=OP.max)
            eq = tmp(TA)
            nc.vector.tensor_tensor(out=_v(eq[:], 0, [[A, T], [1, A]]),
                                    in0=_v(iou[:], 0, [[A, T], [1, A]]),
                                    in1=_v(rmax[:], 0, [[1, T], [0, A]]),
                                    op=OP.is_equal)
            fval = tmp(TA)
            nc.vector.tensor_tensor(out=_v(fval[:], 0, [[A, T], [1, A]]),
                                    in0=_v(eq[:], 0, [[A, T], [1, A]]),
                                    in1=_v(fp[:], O_WCONST, [[0, T], [1, A]]),
                                    op=OP.mult)
            m2 = tmp(T)
            nc.vector.tensor_reduce(out=m2[:],
                                    in_=_v(fval[:], 0, [[A, T], [1, A]]),
                                    axis=AX.X, op=OP.max)
            fmask = tmp(TA)
            nc.vector.tensor_tensor(out=_v(fmask[:], 0, [[A, T], [1, A]]),
                                    in0=_v(fval[:], 0, [[A, T], [1, A]]),
                                    in1=_v(m2[:], 0, [[1, T], [0, A]]),
                                    op=OP.is_equal)

            # cls: se = sum_j e, lse via log2 bit trick, ce = lse - s  (q5)
            se = tmp(TA)
            nc.vector.tensor_reduce(out=_v(se[:], 0, [[A, T], [1, A]]),
                                    in_=_v(e[:], 0, [[100, T], [NCLS, A], [1, NCLS]]),
                                    axis=AX.X, op=OP.add)
            lgf = tmp(TA)
            nc.vector.tensor_copy(out=lgf[:], in_=se[:].bitcast(i32))
            lse = tmp(TA)
            nc.vector.tensor_scalar(out=lse[:], in0=lgf[:],
                                    scalar1=LN2 / (1 << 23),
                                    scalar2=-LOG_BIAS * LN2,
                                    op0=OP.mult, op1=OP.add)
            nc.vector.tensor_tensor(out=_v(SRC[:], 5 * TA, [[A, T], [1, A]]),
                                    in0=_v(lse[:], 0, [[A, T], [1, A]]),
                                    in1=_v(fp[:], O_SAUX, [[A, T], [1, A]]),
                                    op=OP.subtract)

            # best-anchor selection of all six quantities in one mul+reduce
            selm = pool.tile([P, 6 * TA], f32, name="selm")
            nc.vector.tensor_tensor(out=_v(selm[:], 0, [[TA, 6], [1, TA]]),
                                    in0=_v(SRC[:], 0, [[TA, 6], [1, TA]]),
                                    in1=_v(fmask[:], 0, [[0, 6], [1, TA]]),
                                    op=OP.mult)
            selq = pool.tile([P, 6 * T], f32, name="selq")
            nc.vector.tensor_reduce(out=_v(selq[:], 0, [[T, 6], [1, T]]),
                                    in_=_v(selm[:], 0, [[TA, 6], [A, T], [1, A]]),
                                    axis=AX.X, op=OP.add)

            def ttr(in0, in1, col, n):
                junk = tmp(n)
                nc.vector.tensor_tensor(out=junk[:], in0=in0, in1=in1,
                                        op=OP.mult)
                nc.vector.tensor_reduce(out=_v(partials[:], col, [[1, 1]]),
                                        in_=junk[:], axis=AX.X, op=OP.add)

            # box loss: sum oscl4 * (sel - aux4)^2 -> partials[0]
            d4 = tmp(4 * T)
            nc.vector.tensor_tensor(out=d4[:], in0=_v(selq[:], T, [[1, 4 * T]]),
                                    in1=_v(fp[:], O_AUX4, [[1, 4 * T]]),
                                    op=OP.subtract)
            d4m = tmp(4 * T)
            nc.vector.tensor_tensor(out=d4m[:], in0=d4[:],
                                    in1=_v(fp[:], O_OSCL4, [[1, 4 * T]]),
                                    op=OP.mult)
            ttr(d4m[:], d4[:], 0, 4 * T)

            # conf terms from u_sel: sum obj*u^2 -> [1], sum obj*u -> [2]
            OBJv = _v(fp[:], O_OBJ, [[1, T]])
            um = tmp(T)
            nc.vector.tensor_tensor(out=um[:], in0=_v(selq[:], 0, [[1, T]]),
                                    in1=OBJv, op=OP.mult)
            ttr(um[:], _v(selq[:], 0, [[1, T]]), 1, T)
            ttr(um[:], OBJv, 2, T)

            # cls: sum obj * ce_sel -> [3]
            ttr(_v(selq[:], 5 * T, [[1, T]]), OBJv, 3, T)

            nc.sync.dma_start(out=partials_d[:], in_=partials[:])

    if split:
        _split_multi_waits(nc)
    return nc


# -------------------------------------------------------------- shard builder
def _make_in_maps(out, gt_boxes, anchor_np, gt_classes_np, num_box_np, T):
    obj, xo, yo, tw, th, cls_t = _build_target_np(gt_boxes, gt_classes_np,
                                                  num_box_np)
    SLOTS = P * T
    TA = T * A
    out_r = out.reshape(B, A, 25, HWC)
    sqa = np.sqrt(anchor_np)                       # [A, 2]

    in_maps = []
    for c in range(CORES):
        sl = slice(c * BC, (c + 1) * BC)
        ob = obj[sl]                               # [BC, HWC]
        bloc, hwloc = np.nonzero(ob > 0)
        K = len(bloc)
        assert K <= SLOTS

        def place(vals):
            buf = np.zeros(SLOTS, dtype=np.float32)
            buf[:K] = vals
            return buf.reshape(P, T)

        objv = place(np.ones(K, dtype=np.float32))
        xov = place(xo[sl][bloc, hwloc])
        yov = place(yo[sl][bloc, hwloc])
        twv = place(tw[sl][bloc, hwloc])
        thv = place(th[sl][bloc, hwloc])

        # occupied-cell prediction channels [K, A, 25] -> chans 20..24 f32,
        # logits 0..19 bf16
        colsb = np.zeros((SLOTS, A, 25), dtype=np.float32)
        if K:
            colsb[:K] = out_r[sl].transpose(0, 3, 1, 2)[bloc, hwloc]
        cols_xw = colsb[:, :, 20:25].reshape(P, T, A, 5)        # (t,a,{c,x,y,w,h})
        # fpack wants (t, 5ch, a): chan-major within cell
        cols_xw = np.ascontiguousarray(
            cols_xw.transpose(0, 1, 3, 2)).reshape(P, 25 * T)
        logits = np.ascontiguousarray(
            colsb[:, :, :20]).reshape(P, 100 * T)               # (t,a,j)

        # target-class logit per (t, a)
        clsv = place(cls_t[sl][bloc, hwloc].astype(np.float32)).astype(np.int64)
        s_aux = np.take_along_axis(
            colsb[:, :, :20].reshape(SLOTS, A, 20),
            clsv.reshape(SLOTS, 1, 1).repeat(A, axis=1), axis=2
        )[:, :, 0].reshape(P, TA).astype(np.float32)

        # xi-space target box edges (t, {x,y}): center 2o-1, half-width t_wh
        cxv = 2.0 * xov - 1.0
        cyv = 2.0 * yov - 1.0
        b1 = np.stack([cxv - twv, cyv - thv], axis=1).reshape(P, 2 * T)
        b2 = np.stack([cxv + twv, cyv + thv], axis=1).reshape(P, 2 * T)
        tarea = (twv * thv).reshape(P, T)

        wconst = np.broadcast_to(A - np.arange(A, dtype=np.float32), (P, A))
        sqav = np.broadcast_to(sqa.reshape(1, 10), (P, 10))

        aux4 = np.stack([cxv, cyv, np.sqrt(twv), np.sqrt(thv)],
                        axis=1).reshape(P, 4 * T)
        oscl4 = np.stack([0.25 * objv, 0.25 * objv, objv, objv],
                         axis=1).reshape(P, 4 * T)

        fpack = np.concatenate(
            [cols_xw, b1, b2, tarea, wconst, sqav,
             s_aux, aux4, oscl4, objv.reshape(P, T)], axis=1)

        # dense conf channels: [BC, A, HWC] -> [P, 1280] bf16
        confd = out_r[sl][:, :, 20, :].reshape(P, -1)

        in_maps.append({
            "fpack": np.ascontiguousarray(fpack, dtype=np.float32),
            "lgpack": _bf16(logits),
            "confd": _bf16(confd),
        })
    return in_maps


def _combine(results, ks):
    box_s = confu2 = confu1 = cls_s = 0.0
    du2 = du1 = 0.0
    for c in range(CORES):
        pr = results[c]["partials"].astype(np.float64)
        box_s += pr[:, 0].sum()
        confu2 += pr[:, 1].sum()
        confu1 += pr[:, 2].sum()
        cls_s += pr[:, 3].sum()
        du2 += pr[:, 4].sum()
        du1 += pr[:, 5].sum()
    dense = 0.25 * (float(B * A * HWC) + 2.0 * du1 + du2)
    K = float(sum(ks))
    # conf: 0.25*sum obj*(u-1)^2 ; noobj corr: 0.25*sum obj*(u+1)^2
    conf_s = 0.25 * (confu2 - 2.0 * confu1 + K)
    nob_c = 0.25 * (confu2 + 2.0 * confu1 + K)
    box_loss = np.float32(LAM_COORD / B * box_s)
    conf_loss = np.float32(LAM_OBJ / B * conf_s)
    noobj_loss = np.float32(LAM_NOOBJ / B * (dense - nob_c))
    cls_loss = np.float32(LAM_CLS / B * cls_s)
    return (box_loss, conf_loss, noobj_loss, cls_loss)


# ---------------------------------------------------------------- entry point
def kernel(out, gt_boxes, anchor, gt_classes, num_box):
    from concourse.bass_utils import run_bass_kernel_spmd

    out = np.ascontiguousarray(np.asarray(out, dtype=np.float32))
    gt_boxes = np.asarray(gt_boxes, dtype=np.float32)
    anchor_np = np.asarray(anchor, dtype=np.float32)
    gt_classes_np = np.asarray(gt_classes)
    num_box_np = np.asarray(num_box)

    # per-core occupied-cell counts decide the compiled tile factor T
    obj = _build_target_np(gt_boxes, gt_classes_np, num_box_np)[0]
    ks = [int((obj[c * BC:(c + 1) * BC] > 0).sum()) for c in range(CORES)]
    maxk = max(ks)
    T = 9 if maxk <= 9 * P else 13
    assert maxk <= 13 * P

    in_maps = _make_in_maps(out, gt_boxes, anchor_np, gt_classes_np,
                            num_box_np, T)

    import os
    key = f"nc{T}"
    if key not in _CACHE:
        _CACHE[key] = _build_nc(T)
    trace = os.environ.get("KERNEL_TRACE", "0") == "1"
    res = run_bass_kernel_spmd(_CACHE[key], in_maps, core_ids=list(range(CORES)),
                               trace=trace)
    if trace:
        print(f"HW exec time: {res.exec_time_ns} ns  (mean {res.mean_exec_time_ns})")
    return _combine(res.results, ks)
